# revision 10
# baseline (speedup 1.0000x reference)
"""MoE feed-forward (8 experts, top-2, D=1024, H=4096) on 8 Trainium2 cores.

Strategy: expert-parallel with host-side routing.
  - Host computes the gating (logits -> top-2 -> softmax) in fp64 and
    gathers each expert's tokens into a padded, transposed activation
    matrix (C = padded per-expert token capacity), cast bf16.
  - Core e runs the dense FFN for expert e only over its routed tokens:
        y = gelu(x @ w1[e] + b1[e]) @ w2[e]
    All matmul operands are bf16 (PSUM accumulation stays fp32). The
    hidden dim is processed in 4 quarters of 1024; each quarter's weights
    are SBUF-resident and double-buffered so the next quarter's weights
    stream during the current quarter's compute. The output is
    accumulated across quarters in an SBUF fp32 accumulator (DVE adds
    from PSUM) and written to HBM exactly once.
  - All DMA'd tensors are pre-arranged on the host into
    partition-major blocks so every dma_start is one contiguous run per
    partition (128 descriptors); weight/bias/output DMAs go through the
    Activation HWDGE queue, activation loads through the SP queue.
  - Host combines: out[tok] += p_e * (y[tok] + b2[e]).

Self-contained: hardcodes all shapes from the problem spec.
"""

import numpy as np
import ml_dtypes

import concourse.bass as bass
import concourse.mybir as mybir
import concourse.tile as tile
from concourse.bass_utils import run_bass_kernel_spmd

F32 = mybir.dt.float32
BF16 = mybir.dt.bfloat16
NP_BF16 = ml_dtypes.bfloat16

D_MODEL = 1024
HIDDEN = 4096
N_EXPERTS = 8
TOP_K = 2
NQ = 4                    # hidden-dim quarters
HQ = HIDDEN // NQ         # 1024 hidden units per quarter
DBLK = D_MODEL // 128     # 8
JQ = HQ // 128            # 8
CT = 512                  # max token tile (matmul moving free dim)


def _ct_tiles(C):
    """Token tiles: multiples of 128 covering C, at most CT wide."""
    assert C % 128 == 0
    tiles = []
    off = 0
    while off < C:
        w = min(CT, C - off)
        tiles.append((off, w))
        off += w
    return tiles


# ---------------------------------------------------------------------------
# Walrus workaround: this container's Tile emits instructions carrying more
# sync waits than the bundled walrus accepts ("Too many sync wait commands").
# Hoist excess waits onto EventSemaphore instructions placed immediately
# before the overloaded instruction (same engine, same block) — semantically
# identical: the engine blocks on each wait in program order.
_CAP_BY_OPCODE = {"EventSemaphore": 2}
_DEFAULT_CAP = 1
_split_counter = [0]


def split_excess_waits(nc):
    for f in nc.m.functions:
        for bb in f.blocks:
            new_insts = []
            changed = False
            for inst in bb.instructions:
                si = inst.sync_info
                waits = list(si.on_wait) if si is not None else []
                cap = _CAP_BY_OPCODE.get(inst.opcode, _DEFAULT_CAP)
                if len(waits) > cap:
                    changed = True
                    excess, keep = waits[:-cap], waits[-cap:]
                    for i in range(0, len(excess), 2):
                        _split_counter[0] += 1
                        new_insts.append(mybir.InstEventSemaphore(
                            name=f"I-waitsplit-{_split_counter[0]}",
                            engine=inst.engine,
                            sync_info=mybir.SyncInfo(
                                on_wait=excess[i:i + 2], on_update=[]),
                        ))
                    inst.sync_info = mybir.SyncInfo(
                        on_wait=keep, on_update=list(si.on_update))
                new_insts.append(inst)
            if changed:
                bb.instructions = new_insts
    return nc


# ---------------------------------------------------------------------------
def build_nc(C, act=None, reps=1, bufs_x=3, bufs_h=2, bufs_ps1=2, bufs_ps2=2,
             bufs_y=3, unroll=1):
    """Per-core FFN program. Inputs (pre-arranged partition-major):
         xp [D*C] bf16      — per ct tile: [128p][8d][W] blocks, concat
         w1 [NQ,128,DBLK,HQ] bf16
         w2 [NQ,128,JQ,D]    bf16
         b1 [128,NQ,JQ]      f32
       Output: y [C, D] f32.
    """
    if act is None:
        act = mybir.ActivationFunctionType.Gelu
    tiles = _ct_tiles(C)
    CBLK = C // 128
    nc = bass.Bass()
    xp = nc.dram_tensor("xp", [D_MODEL * C], BF16, kind="ExternalInput")
    w1 = nc.dram_tensor("w1", [NQ, 128, DBLK, HQ], BF16, kind="ExternalInput")
    b1 = nc.dram_tensor("b1", [128, NQ, JQ], F32, kind="ExternalInput")
    w2 = nc.dram_tensor("w2", [NQ, 128, JQ, D_MODEL], BF16,
                        kind="ExternalInput")
    y = nc.dram_tensor("y", [C, D_MODEL], F32, kind="ExternalOutput")

    with tile.TileContext(nc) as tc:
        with (
            tc.tile_pool(name="wpool", bufs=2) as wpool,
            tc.tile_pool(name="xpool", bufs=bufs_x) as xpool,
            tc.tile_pool(name="hpool", bufs=bufs_h) as hpool,
            tc.tile_pool(name="apool", bufs=1) as apool,
            tc.tile_pool(name="ypool", bufs=bufs_y) as ypool,
            tc.tile_pool(name="ps1", bufs=bufs_ps1, space="PSUM") as ps1,
            tc.tile_pool(name="ps2", bufs=bufs_ps2, space="PSUM") as ps2,
        ):
            def whole(_=None):
                yacc = apool.tile([128, CBLK, D_MODEL], F32, tag="yacc")
                b1t = wpool.tile([128, NQ, JQ], F32, tag="b1t")
                nc.scalar.dma_start(out=b1t[:], in_=b1.ap())
                for q in range(NQ):
                    # this quarter's weights (double-buffered across q)
                    w1t = wpool.tile([128, DBLK, HQ], BF16, tag="w1t")
                    nc.scalar.dma_start(out=w1t[:], in_=w1.ap()[q])
                    w2t = wpool.tile([128, JQ, D_MODEL], BF16, tag="w2t")
                    nc.scalar.dma_start(out=w2t[:], in_=w2.ap()[q])

                    for off, W in tiles:
                        xt = xpool.tile([128, DBLK, W], BF16, tag="xt")
                        base = 128 * DBLK * off
                        nc.sync.dma_start(
                            out=xt[:],
                            in_=xp.ap()[base:base + 128 * DBLK * W]
                            .rearrange("(p d c) -> p d c", p=128, d=DBLK))

                        hT = hpool.tile([128, JQ, W], BF16, tag="hT")
                        for j in range(JQ):
                            ps = ps1.tile([128, W], F32, tag="ps")
                            for d in range(DBLK):
                                nc.tensor.matmul(
                                    ps[:],
                                    w1t[:, d, j * 128:(j + 1) * 128],
                                    xt[:, d, :],
                                    start=(d == 0), stop=(d == DBLK - 1))
                            nc.scalar.activation(
                                hT[:, j, :], ps[:], act,
                                bias=b1t[:, q, j:j + 1])

                        for cs in range(W // 128):
                            cb = off // 128 + cs
                            p2 = ps2.tile([128, 2, 512], F32, tag="p2")
                            for dh in range(2):
                                for j in range(JQ):
                                    nc.tensor.matmul(
                                        p2[:, dh, :],
                                        hT[:, j, cs * 128:(cs + 1) * 128],
                                        w2t[:, j, dh * 512:(dh + 1) * 512],
                                        start=(j == 0), stop=(j == JQ - 1))
                            ya = yacc[:, cb, :]
                            if q == 0:
                                nc.vector.tensor_copy(ya, p2[:])
                            elif q < NQ - 1:
                                nc.vector.tensor_add(ya, ya, p2[:])
                            else:
                                yo = ypool.tile([128, 2, 512], F32, tag="yo")
                                nc.vector.tensor_add(yo[:], ya, p2[:])
                                nc.sync.dma_start(
                                    out=y.ap()[off + cs * 128:
                                               off + (cs + 1) * 128, :],
                                    in_=yo[:])

            if reps == 1:
                whole()
            elif reps == unroll:
                for _ in range(reps):
                    whole()
            else:
                assert reps % unroll == 0
                with tc.For_i(0, reps // unroll, 1):
                    for _ in range(unroll):
                        whole()
    return nc


# ---------------------------------------------------------------------------
def _gating(x2d, gate_w, gate_b):
    """fp64 host gating; returns per-expert (idx, prob) matching jax top_k
    (ties -> lower index, measure-zero for random inputs)."""
    logits = x2d.astype(np.float64) @ gate_w.astype(np.float64) \
        + gate_b.astype(np.float64)
    i1 = np.argmax(logits, axis=-1)
    n = len(logits)
    ar = np.arange(n)
    v1 = logits[ar, i1]
    l2 = logits.copy()
    l2[ar, i1] = -np.inf
    i2 = np.argmax(l2, axis=-1)
    v2 = l2[ar, i2]
    m = np.maximum(v1, v2)
    e1 = np.exp(v1 - m)
    e2 = np.exp(v2 - m)
    s = e1 + e2
    p1 = (e1 / s)
    p2 = (e2 / s)
    out = []
    for e in range(N_EXPERTS):
        m1 = i1 == e
        m2 = i2 == e
        idx = np.nonzero(m1 | m2)[0]
        prob = np.where(m1, p1, p2)[idx].astype(np.float32)
        out.append((idx, prob))
    return out


def capacity_for(routes):
    max_n = max(len(idx) for idx, _ in routes)
    return max(CT, -(-max_n // 128) * 128)


def _pack_x(xTe, C):
    """xTe [D, C] f32 -> flat [D*C] bf16 in per-tile [128p][8d][W] blocks."""
    parts = []
    for off, W in _ct_tiles(C):
        blk = xTe[:, off:off + W].reshape(DBLK, 128, W).transpose(1, 0, 2)
        parts.append(np.ascontiguousarray(blk).reshape(-1))
    return np.concatenate(parts).astype(NP_BF16)


def make_in_maps(x2d, routes, w1, b1, w2, C):
    in_maps = []
    for e in range(N_EXPERTS):
        idx, _ = routes[e]
        xTe = np.zeros((D_MODEL, C), dtype=np.float32)
        xTe[:, :len(idx)] = x2d[idx].T
        w1p = w1[e].reshape(DBLK, 128, NQ, HQ).transpose(2, 1, 0, 3)
        w2p = w2[e].reshape(NQ, JQ, 128, D_MODEL).transpose(0, 2, 1, 3)
        b1p = b1[e].reshape(NQ, JQ, 128).transpose(2, 0, 1)
        in_maps.append({
            "xp": _pack_x(xTe, C),
            "w1": np.ascontiguousarray(w1p).astype(NP_BF16),
            "b1": np.ascontiguousarray(b1p).astype(np.float32),
            "w2": np.ascontiguousarray(w2p).astype(NP_BF16),
        })
    return in_maps


_NC_CACHE = {}


def kernel(x, gate_w, gate_b, w1, b1, w2, b2):
    x = np.asarray(x, dtype=np.float32)
    gate_w = np.asarray(gate_w, dtype=np.float32)
    gate_b = np.asarray(gate_b, dtype=np.float32)
    w1 = np.asarray(w1, dtype=np.float32)
    b1 = np.asarray(b1, dtype=np.float32)
    w2 = np.asarray(w2, dtype=np.float32)
    b2 = np.asarray(b2, dtype=np.float32)

    B, T, D = x.shape
    x2d = x.reshape(-1, D)
    routes = _gating(x2d, gate_w, gate_b)
    C = capacity_for(routes)

    if C not in _NC_CACHE:
        nc = build_nc(C)
        split_excess_waits(nc)
        _NC_CACHE[C] = nc
    nc = _NC_CACHE[C]

    in_maps = make_in_maps(x2d, routes, w1, b1, w2, C)
    res = run_bass_kernel_spmd(nc, in_maps, core_ids=list(range(N_EXPERTS)))

    out2d = np.zeros((B * T, D_MODEL), dtype=np.float32)
    for e in range(N_EXPERTS):
        idx, prob = routes[e]
        n = len(idx)
        y_e = res.results[e]["y"][:n] + b2[e]
        out2d[idx] += prob[:, None] * y_e
    return out2d.reshape(B, T, D_MODEL)


# revision 11
# speedup vs baseline: 1.0582x; 1.0582x over previous
"""MoE feed-forward (8 experts, top-2, D=1024, H=4096) on 8 Trainium2 cores.

Strategy: expert-parallel with host-side routing.
  - Host computes the gating (logits -> top-2 -> softmax) in fp64 and
    gathers each expert's tokens into a padded, transposed activation
    matrix (C = padded per-expert token capacity), cast bf16.
  - Core e runs the dense FFN for expert e only over its routed tokens:
        y = gelu(x @ w1[e] + b1[e]) @ w2[e]
    All matmul operands are bf16 (PSUM accumulation stays fp32). The
    hidden dim is processed in 4 quarters of 1024; each quarter's weights
    are SBUF-resident and double-buffered so the next quarter's weights
    stream during the current quarter's compute. The output is
    accumulated across quarters in an SBUF fp32 accumulator (DVE adds
    from PSUM) and written to HBM exactly once.
  - All DMA'd tensors are pre-arranged on the host into
    partition-major blocks so every dma_start is one contiguous run per
    partition (128 descriptors); weight/bias/output DMAs go through the
    Activation HWDGE queue, activation loads through the SP queue.
  - Host combines: out[tok] += p_e * (y[tok] + b2[e]).

Self-contained: hardcodes all shapes from the problem spec.
"""

import numpy as np
import ml_dtypes

import concourse.bass as bass
import concourse.mybir as mybir
import concourse.tile as tile
from concourse.bass_utils import run_bass_kernel_spmd

F32 = mybir.dt.float32
BF16 = mybir.dt.bfloat16
NP_BF16 = ml_dtypes.bfloat16

D_MODEL = 1024
HIDDEN = 4096
N_EXPERTS = 8
TOP_K = 2
NQ = 4                    # hidden-dim quarters
HQ = HIDDEN // NQ         # 1024 hidden units per quarter
DBLK = D_MODEL // 128     # 8
JQ = HQ // 128            # 8
CT = 512                  # max token tile (matmul moving free dim)


def _ct_tiles(C):
    """Token tiles: multiples of 128 covering C, at most CT wide."""
    assert C % 128 == 0
    tiles = []
    off = 0
    while off < C:
        w = min(CT, C - off)
        tiles.append((off, w))
        off += w
    return tiles


# ---------------------------------------------------------------------------
# Walrus workaround: this container's Tile emits instructions carrying more
# sync waits than the bundled walrus accepts ("Too many sync wait commands").
# Hoist excess waits onto EventSemaphore instructions placed immediately
# before the overloaded instruction (same engine, same block) — semantically
# identical: the engine blocks on each wait in program order.
_CAP_BY_OPCODE = {"EventSemaphore": 2}
_DEFAULT_CAP = 1
_split_counter = [0]


def split_excess_waits(nc):
    for f in nc.m.functions:
        for bb in f.blocks:
            new_insts = []
            changed = False
            for inst in bb.instructions:
                si = inst.sync_info
                waits = list(si.on_wait) if si is not None else []
                cap = _CAP_BY_OPCODE.get(inst.opcode, _DEFAULT_CAP)
                if len(waits) > cap:
                    changed = True
                    excess, keep = waits[:-cap], waits[-cap:]
                    for i in range(0, len(excess), 2):
                        _split_counter[0] += 1
                        new_insts.append(mybir.InstEventSemaphore(
                            name=f"I-waitsplit-{_split_counter[0]}",
                            engine=inst.engine,
                            sync_info=mybir.SyncInfo(
                                on_wait=excess[i:i + 2], on_update=[]),
                        ))
                    inst.sync_info = mybir.SyncInfo(
                        on_wait=keep, on_update=list(si.on_update))
                new_insts.append(inst)
            if changed:
                bb.instructions = new_insts
    return nc


# ---------------------------------------------------------------------------
def build_nc(C, act=None, reps=1, bufs_x=3, bufs_h=2, bufs_ps1=2, bufs_ps2=2,
             bufs_y=3, unroll=1):
    """Per-core FFN program. Inputs (pre-arranged partition-major):
         xp [D*C] bf16      — per ct tile: [128p][8d][W] blocks, concat
         w1 [NQ,128,DBLK,HQ] bf16
         w2 [NQ,128,JQ,D]    bf16
         b1 [128,NQ,JQ]      f32
       Output: y [C, D] f32.
    """
    if act is None:
        act = mybir.ActivationFunctionType.Gelu
    tiles = _ct_tiles(C)
    CBLK = C // 128
    nc = bass.Bass()
    xp = nc.dram_tensor("xp", [D_MODEL * C], BF16, kind="ExternalInput")
    w1 = nc.dram_tensor("w1", [NQ, 128, DBLK, HQ], BF16, kind="ExternalInput")
    b1 = nc.dram_tensor("b1", [128, NQ, JQ], F32, kind="ExternalInput")
    w2 = nc.dram_tensor("w2", [NQ, 128, JQ, D_MODEL], BF16,
                        kind="ExternalInput")
    y = nc.dram_tensor("y", [C, D_MODEL], F32, kind="ExternalOutput")

    with tile.TileContext(nc) as tc:
        with (
            tc.tile_pool(name="wpool", bufs=2) as wpool,
            tc.tile_pool(name="xpool", bufs=bufs_x) as xpool,
            tc.tile_pool(name="hpool", bufs=bufs_h) as hpool,
            tc.tile_pool(name="apool", bufs=1) as apool,
            tc.tile_pool(name="ypool", bufs=bufs_y) as ypool,
            tc.tile_pool(name="ps1", bufs=bufs_ps1, space="PSUM") as ps1,
            tc.tile_pool(name="ps2", bufs=bufs_ps2, space="PSUM") as ps2,
        ):
            def whole(_=None):
                yacc = apool.tile([128, CBLK, D_MODEL], F32, tag="yacc")
                b1t = wpool.tile([128, NQ, JQ], F32, tag="b1t")
                nc.scalar.dma_start(out=b1t[:], in_=b1.ap())
                for q in range(NQ):
                    # this quarter's weights (double-buffered across q)
                    w1t = wpool.tile([128, DBLK, HQ], BF16, tag="w1t")
                    nc.scalar.dma_start(out=w1t[:], in_=w1.ap()[q])
                    w2t = wpool.tile([128, JQ, D_MODEL], BF16, tag="w2t")
                    nc.scalar.dma_start(out=w2t[:], in_=w2.ap()[q])

                    for off, W in tiles:
                        xt = xpool.tile([128, DBLK, W], BF16, tag="xt")
                        base = 128 * DBLK * off
                        nc.sync.dma_start(
                            out=xt[:],
                            in_=xp.ap()[base:base + 128 * DBLK * W]
                            .rearrange("(p d c) -> p d c", p=128, d=DBLK))

                        hT = hpool.tile([128, JQ, W], BF16, tag="hT")
                        for j in range(JQ):
                            ps = ps1.tile([128, W], F32, tag="ps")
                            for d in range(DBLK):
                                nc.tensor.matmul(
                                    ps[:],
                                    w1t[:, d, j * 128:(j + 1) * 128],
                                    xt[:, d, :],
                                    start=(d == 0), stop=(d == DBLK - 1))
                            nc.scalar.activation(
                                hT[:, j, :], ps[:], act,
                                bias=b1t[:, q, j:j + 1])

                        for cs in range(W // 128):
                            cb = off // 128 + cs
                            p2 = ps2.tile([128, 2, 512], F32, tag="p2")
                            for dh in range(2):
                                for j in range(JQ):
                                    nc.tensor.matmul(
                                        p2[:, dh, :],
                                        hT[:, j, cs * 128:(cs + 1) * 128],
                                        w2t[:, j, dh * 512:(dh + 1) * 512],
                                        start=(j == 0), stop=(j == JQ - 1))
                            ya = yacc[:, cb, :]
                            if q == 0:
                                nc.vector.tensor_copy(ya, p2[:])
                            elif q < NQ - 1:
                                nc.vector.tensor_add(ya, ya, p2[:])
                            else:
                                yo = ypool.tile([128, 2, 512], F32, tag="yo")
                                nc.vector.tensor_add(yo[:], ya, p2[:])
                                nc.scalar.dma_start(
                                    out=y.ap()[off + cs * 128:
                                               off + (cs + 1) * 128, :],
                                    in_=yo[:])

            if reps == 1:
                whole()
            elif reps == unroll:
                for _ in range(reps):
                    whole()
            else:
                assert reps % unroll == 0
                with tc.For_i(0, reps // unroll, 1):
                    for _ in range(unroll):
                        whole()
    return nc


# ---------------------------------------------------------------------------
def _gating(x2d, gate_w, gate_b):
    """fp64 host gating; returns per-expert (idx, prob) matching jax top_k
    (ties -> lower index, measure-zero for random inputs)."""
    logits = x2d.astype(np.float64) @ gate_w.astype(np.float64) \
        + gate_b.astype(np.float64)
    i1 = np.argmax(logits, axis=-1)
    n = len(logits)
    ar = np.arange(n)
    v1 = logits[ar, i1]
    l2 = logits.copy()
    l2[ar, i1] = -np.inf
    i2 = np.argmax(l2, axis=-1)
    v2 = l2[ar, i2]
    m = np.maximum(v1, v2)
    e1 = np.exp(v1 - m)
    e2 = np.exp(v2 - m)
    s = e1 + e2
    p1 = (e1 / s)
    p2 = (e2 / s)
    out = []
    for e in range(N_EXPERTS):
        m1 = i1 == e
        m2 = i2 == e
        idx = np.nonzero(m1 | m2)[0]
        prob = np.where(m1, p1, p2)[idx].astype(np.float32)
        out.append((idx, prob))
    return out


def capacity_for(routes):
    max_n = max(len(idx) for idx, _ in routes)
    return max(CT, -(-max_n // 128) * 128)


def _pack_x(xTe, C):
    """xTe [D, C] f32 -> flat [D*C] bf16 in per-tile [128p][8d][W] blocks."""
    parts = []
    for off, W in _ct_tiles(C):
        blk = xTe[:, off:off + W].reshape(DBLK, 128, W).transpose(1, 0, 2)
        parts.append(np.ascontiguousarray(blk).reshape(-1))
    return np.concatenate(parts).astype(NP_BF16)


def make_in_maps(x2d, routes, w1, b1, w2, C):
    in_maps = []
    for e in range(N_EXPERTS):
        idx, _ = routes[e]
        xTe = np.zeros((D_MODEL, C), dtype=np.float32)
        xTe[:, :len(idx)] = x2d[idx].T
        w1p = w1[e].reshape(DBLK, 128, NQ, HQ).transpose(2, 1, 0, 3)
        w2p = w2[e].reshape(NQ, JQ, 128, D_MODEL).transpose(0, 2, 1, 3)
        b1p = b1[e].reshape(NQ, JQ, 128).transpose(2, 0, 1)
        in_maps.append({
            "xp": _pack_x(xTe, C),
            "w1": np.ascontiguousarray(w1p).astype(NP_BF16),
            "b1": np.ascontiguousarray(b1p).astype(np.float32),
            "w2": np.ascontiguousarray(w2p).astype(NP_BF16),
        })
    return in_maps


_NC_CACHE = {}


def kernel(x, gate_w, gate_b, w1, b1, w2, b2):
    x = np.asarray(x, dtype=np.float32)
    gate_w = np.asarray(gate_w, dtype=np.float32)
    gate_b = np.asarray(gate_b, dtype=np.float32)
    w1 = np.asarray(w1, dtype=np.float32)
    b1 = np.asarray(b1, dtype=np.float32)
    w2 = np.asarray(w2, dtype=np.float32)
    b2 = np.asarray(b2, dtype=np.float32)

    B, T, D = x.shape
    x2d = x.reshape(-1, D)
    routes = _gating(x2d, gate_w, gate_b)
    C = capacity_for(routes)

    if C not in _NC_CACHE:
        nc = build_nc(C)
        split_excess_waits(nc)
        _NC_CACHE[C] = nc
    nc = _NC_CACHE[C]

    in_maps = make_in_maps(x2d, routes, w1, b1, w2, C)
    res = run_bass_kernel_spmd(nc, in_maps, core_ids=list(range(N_EXPERTS)))

    out2d = np.zeros((B * T, D_MODEL), dtype=np.float32)
    for e in range(N_EXPERTS):
        idx, prob = routes[e]
        n = len(idx)
        y_e = res.results[e]["y"][:n] + b2[e]
        out2d[idx] += prob[:, None] * y_e
    return out2d.reshape(B, T, D_MODEL)
